# revision 21
# baseline (speedup 1.0000x reference)
"""Trainium2 Bass kernel for CSR-sparse-weight linear layer.

Computes out[b,s,m] = sum_h x[b,s,h] * W[m,h] where W is given in CSR form
(values, col_idx, row_ptr), M = H = 4096, 50% density.

Strategy: decode CSR -> dense W on host (O(NNZ), trivial next to the GEMM),
shard x data-parallel across 8 NeuronCores along the flattened batch*seq dim
(8192 rows -> 1024 rows/core), replicate W. Each core computes
out_shard^T = W @ x_shard^T as a tiled matmul on the tensor engine:
contraction dim H on SBUF partitions, W tiles stationary, x moving.
"""

import os
import sys

sys.path.insert(0, "/opt/trn_rl_repo")

import numpy as np

# Problem shapes (hardcoded per harness contract)
B, S, H, M = 4, 2048, 4096, 4096
NTOT = B * S            # 8192 flattened rows
NCORES = 8
N = NTOT // NCORES      # 1024 rows per core
P = 128                 # SBUF partitions
KO = H // P             # 32 contraction tiles
MO = M // P             # 32 output-feature tiles
NF = 512                # moving free dim per matmul (= 1 PSUM bank of fp32)

_CACHE = {}


def _dtype_knob():
    return os.environ.get("BASS_KERNEL_DTYPE", "f32r")


def _variant_knob():
    return os.environ.get("BASS_KERNEL_VARIANT", "simple")


def _build_nc(knob, reps=1, variant=None):
    import concourse.mybir as mybir
    import concourse.tile as tile
    from concourse import bacc

    if variant is None:
        variant = _variant_knob()
    f32 = mybir.dt.float32
    mm_dt = {"f32r": mybir.dt.float32r, "f32": f32, "bf16": mybir.dt.bfloat16}[knob]
    wire_dt = mm_dt

    nc = bacc.Bacc("TRN2", target_bir_lowering=False, debug=False)

    # xT[p, ko, n] = x_shard[n, ko*128 + p]
    xT_d = nc.dram_tensor("xT", [P, KO, N], wire_dt, kind="ExternalInput")
    # wT[mo, p, ko, j] = W[mo*128 + j, ko*128 + p]
    wT_d = nc.dram_tensor("wT", [MO, P, KO, P], wire_dt, kind="ExternalInput")
    # out[p, mo, n] = out_shard[n, mo*128 + p]
    out_d = nc.dram_tensor("out", [P, MO, N], f32, kind="ExternalOutput")

    with tile.TileContext(nc) as tc:
        with (
            tc.tile_pool(name="xpool", bufs=1) as xpool,
            tc.tile_pool(name="wpool", bufs=3) as wpool,
            tc.tile_pool(name="opool", bufs=4) as opool,
            tc.tile_pool(name="pspool", bufs=4, space="PSUM") as pspool,
        ):

            def load_x_stripe(x_sb, nf):
                for ko in range(KO):
                    nc.sync.dma_start(
                        x_sb[:, ko, nf * NF : (nf + 1) * NF],
                        xT_d[:, ko, nf * NF : (nf + 1) * NF],
                    )

            def fetch_w(mo):
                w_sb = wpool.tile([P, KO, P], mm_dt, tag="w")
                nc.sync.dma_start(w_sb[:], wT_d[mo])
                return w_sb

            def group(w_sb, x_sb, mo, nf):
                ps = pspool.tile([P, NF], f32)
                for ko in range(KO):
                    nc.tensor.matmul(
                        ps[:],
                        w_sb[:, ko, :],
                        x_sb[:, ko, nf * NF : (nf + 1) * NF],
                        start=(ko == 0),
                        stop=(ko == KO - 1),
                    )
                o_sb = opool.tile([P, NF], f32)
                nc.vector.tensor_copy(o_sb[:], ps[:])
                nc.sync.dma_start(out_d[:, mo, nf * NF : (nf + 1) * NF], o_sb[:])

            def body_simple(x_sb, w0_sb=None):
                for mo in range(MO):
                    w_sb = w0_sb if (mo == 0 and w0_sb is not None) else fetch_w(mo)
                    for nf in range(N // NF):
                        group(w_sb, x_sb, mo, nf)

            # Phased variant: during the x load the W stream must not starve,
            # so the nf=1 groups of the first PHASE_A m-tiles are postponed to
            # the end (their W tiles re-fetched), and the x nf=1 stripes are
            # loaded only at the start of phase B. All x writes stay BEFORE
            # every group that reads them in trace order -- emitting a read
            # before the write means Tile sees no RAW dep and the result is
            # garbage (measured rel err 0.38 on HW with the naive deferral).
            PHASE_A = 7

            def body_phased(x_sb, w0_sb=None, x_preloaded=False):
                assert N // NF == 2
                for mo in range(PHASE_A):
                    w_sb = w0_sb if (mo == 0 and w0_sb is not None) else fetch_w(mo)
                    group(w_sb, x_sb, mo, 0)
                if not x_preloaded:
                    load_x_stripe(x_sb, 1)
                for mo in range(PHASE_A, MO):
                    w_sb = fetch_w(mo)
                    group(w_sb, x_sb, mo, 0)
                    group(w_sb, x_sb, mo, 1)
                for mo in range(PHASE_A):
                    w_sb = fetch_w(mo)
                    group(w_sb, x_sb, mo, 1)

            if reps == 1:
                # Head interleave (modeled 491us vs 509us plain): DMA order
                # w0, x nf=0, w1, x nf=1; groups (0,0),(1,0),(0,1),(1,1) so
                # the PE has nf=0 work for two m-tiles while the nf=1 stripes
                # are still in flight. w0/w1 stay live across 3 groups --
                # fits wpool bufs=3. All x stripe writes precede every group
                # in trace order (Tile deps are trace-order; violating this
                # returned garbage, rel err 0.38).
                x_sb = xpool.tile([P, KO, N], mm_dt)
                w0_sb = wpool.tile([P, KO, P], mm_dt, tag="w", name="w_head0")
                nc.sync.dma_start(w0_sb[:], wT_d[0])
                load_x_stripe(x_sb, 0)
                w1_sb = wpool.tile([P, KO, P], mm_dt, tag="w", name="w_head1")
                nc.sync.dma_start(w1_sb[:], wT_d[1])
                load_x_stripe(x_sb, 1)
                if variant == "phased":
                    body_phased(x_sb, w0_sb, x_preloaded=True)
                else:
                    group(w0_sb, x_sb, 0, 0)
                    group(w1_sb, x_sb, 1, 0)
                    group(w0_sb, x_sb, 0, 1)
                    group(w1_sb, x_sb, 1, 1)
                    for mo in range(2, MO):
                        w_sb = fetch_w(mo)
                        for nf in range(N // NF):
                            group(w_sb, x_sb, mo, nf)
            else:
                # Measurement mode: x loaded once outside the loop; the loop
                # body is the steady-state W-stream + matmul + store pipeline.
                x_sb = xpool.tile([P, KO, N], mm_dt)
                for nf in range(N // NF):
                    load_x_stripe(x_sb, nf)
                with tc.For_i(0, reps, 1) as i:
                    if variant == "phased":
                        body_phased(x_sb, None, x_preloaded=True)
                    else:
                        body_simple(x_sb, None)
    nc.compile()
    return nc


def _get_nc(knob, reps=1):
    key = (knob, reps, _variant_knob())
    if key not in _CACHE:
        _CACHE[key] = _build_nc(knob, reps)
    return _CACHE[key]


class _Runner:
    """Mirrors bass2jax.run_bass_via_pjrt but keeps sharded inputs on device
    and supports timing repeated executions."""

    def __init__(self, nc, n_cores):
        import concourse.mybir as mybir
        import jax
        from concourse import bass2jax as b2j
        from jax.experimental.shard_map import shard_map
        from jax.sharding import Mesh, PartitionSpec

        b2j.install_neuronx_cc_hook()
        self.jax = jax
        self.n_cores = n_cores

        partition_name = (
            nc.partition_id_tensor.name if nc.partition_id_tensor else None
        )
        in_names, out_names, out_avals, zero_outs = [], [], [], []
        for alloc in nc.m.functions[0].allocations:
            if not isinstance(alloc, mybir.MemoryLocationSet):
                continue
            name = alloc.memorylocations[0].name
            if alloc.kind == "ExternalInput":
                if name != partition_name:
                    in_names.append(name)
            elif alloc.kind == "ExternalOutput":
                shape = tuple(alloc.tensor_shape)
                dtype = mybir.dt.np(alloc.dtype)
                out_names.append(name)
                out_avals.append(jax.core.ShapedArray(shape, dtype))
                zero_outs.append(np.zeros(shape, dtype))
        n_params = len(in_names)
        all_in = list(in_names) + list(out_names)
        if partition_name is not None:
            all_in.append(partition_name)

        def _body(*args):
            operands = list(args)
            if partition_name is not None:
                operands.append(b2j.partition_id_tensor())
            outs = b2j._bass_exec_p.bind(
                *operands,
                out_avals=tuple(out_avals),
                in_names=tuple(all_in),
                out_names=tuple(out_names),
                lowering_input_output_aliases=(),
                sim_require_finite=True,
                sim_require_nnan=True,
                nc=nc,
            )
            return tuple(outs)

        devices = jax.devices()[:n_cores]
        self.mesh = Mesh(np.asarray(devices), ("core",))
        self.pspec = PartitionSpec("core")
        donate = tuple(range(n_params, n_params + len(out_names)))
        in_specs = (self.pspec,) * (n_params + len(out_names))
        out_specs = (self.pspec,) * len(out_names)
        self.fn = jax.jit(
            shard_map(
                _body,
                mesh=self.mesh,
                in_specs=in_specs,
                out_specs=out_specs,
                check_rep=False,
            ),
            donate_argnums=donate,
            keep_unused=True,
        )
        self.in_names = in_names
        self.out_names = out_names
        self.out_avals = out_avals
        self.zero_outs = zero_outs

    def _sharded_put(self, arr):
        from jax.sharding import NamedSharding

        return self.jax.device_put(arr, NamedSharding(self.mesh, self.pspec))

    def put_inputs(self, in_maps):
        concat = [
            np.concatenate([np.asarray(m[name]) for m in in_maps], axis=0)
            for name in self.in_names
        ]
        return [self._sharded_put(a) for a in concat]

    def _zeros(self):
        return [
            self._sharded_put(
                np.zeros((self.n_cores * z.shape[0], *z.shape[1:]), z.dtype)
            )
            for z in self.zero_outs
        ]

    def run(self, dev_inputs):
        outs = self.fn(*dev_inputs, *self._zeros())
        self.jax.block_until_ready(outs)
        return self._split(outs)

    def _split(self, outs):
        return [
            {
                name: np.asarray(outs[i]).reshape(
                    self.n_cores, *self.out_avals[i].shape
                )[c]
                for i, name in enumerate(self.out_names)
            }
            for c in range(self.n_cores)
        ]

    def bench(self, dev_inputs, reps=10):
        import time

        times = []
        outs = None
        for _ in range(reps):
            zouts = self._zeros()
            self.jax.block_until_ready(zouts)
            t0 = time.perf_counter()
            outs = self.fn(*dev_inputs, *zouts)
            self.jax.block_until_ready(outs)
            times.append(time.perf_counter() - t0)
        return self._split(outs), times


def _get_runner(knob, reps=1):
    key = ("runner", knob, reps, _variant_knob())
    if key not in _CACHE:
        _CACHE[key] = _Runner(_get_nc(knob, reps), NCORES)
    return _CACHE[key]


def _decode_csr(values, col_idx, row_ptr):
    counts = np.diff(row_ptr.astype(np.int64))
    row_ids = np.repeat(np.arange(M, dtype=np.int64), counts)
    W = np.zeros((M, H), np.float32)
    W[row_ids, col_idx.astype(np.int64)] = values.astype(np.float32)
    return W


def _prep_in_maps(x, values, col_idx, row_ptr, knob):
    x = np.asarray(x, dtype=np.float32)
    W = _decode_csr(np.asarray(values), np.asarray(col_idx), np.asarray(row_ptr))

    if knob == "bf16":
        import ml_dtypes

        wire = np.dtype(ml_dtypes.bfloat16)
    else:
        wire = np.dtype(np.float32)

    # W[m, h] -> wT[mo, p, ko, j] with m = mo*128+j, h = ko*128+p
    wT = np.ascontiguousarray(
        W.reshape(MO, P, KO, P).transpose(0, 3, 2, 1).astype(wire)
    )
    x_flat = x.reshape(NTOT, H)
    in_maps = []
    for c in range(NCORES):
        xs = x_flat[c * N : (c + 1) * N]                      # [N, H]
        xT = np.ascontiguousarray(
            xs.T.reshape(KO, P, N).transpose(1, 0, 2).astype(wire)
        )                                                     # [P, KO, N]
        in_maps.append({"xT": xT, "wT": wT})
    return in_maps


def _gather_out(results):
    shards = []
    for c in range(NCORES):
        oc = results[c]["out"]                                # [P, MO, N]
        shards.append(oc.transpose(2, 1, 0).reshape(N, M))
    out = np.concatenate(shards, axis=0).reshape(B, S, M)
    return np.ascontiguousarray(out.astype(np.float32))


def kernel(x, values, col_idx, row_ptr):
    from concourse.bass_utils import run_bass_kernel_spmd

    knob = _dtype_knob()
    nc = _get_nc(knob, 1)
    in_maps = _prep_in_maps(x, values, col_idx, row_ptr, knob)
    res = run_bass_kernel_spmd(nc, in_maps, list(range(NCORES)))
    return _gather_out(res.results)


def kernel_bench(x, values, col_idx, row_ptr, reps=10, loop_reps=1):
    """Test-only: returns (output, list of per-call wall times in seconds).
    loop_reps > 1 wraps the whole GEMM in an on-device For_i loop so device
    time dominates the host/RPC overhead; kernel time is then estimated as
    (wall(R) - wall(1)) / (R - 1)."""
    knob = _dtype_knob()
    runner = _get_runner(knob, loop_reps)
    in_maps = _prep_in_maps(x, values, col_idx, row_ptr, knob)
    dev_inputs = runner.put_inputs(in_maps)
    results, times = runner.bench(dev_inputs, reps=reps)
    return _gather_out(results), times
